# revision 1
# baseline (speedup 1.0000x reference)
"""Self-contained TRN2 Bass kernel for nn_BaseVAE loss (8-core SPMD)."""


import math

import numpy as np

import concourse.bass as bass
import concourse.mybir as mybir
from concourse import tile

F32 = mybir.dt.float32
BF16 = mybir.dt.bfloat16
ALU = mybir.AluOpType
ACTF = mybir.ActivationFunctionType
AX = mybir.AxisListType

H = 160
C = 3
NPIX = 480
J = 150
NP_ = 38
NU = 5
NFR = NU * NP_          # 190
EPS = 1e-8
C1 = 1e-4
C2 = 9e-4
CH = [(0, 115), (115, 190)]
JA = 117                # gauss col band split
PA = 30                 # stft pi band split


def make_consts(n=4):
    x = np.arange(11, dtype=np.float64) - 5.0
    g = np.exp(-0.5 * (x / 1.5) ** 2)
    g = g / g.sum()
    gc = np.zeros((160, 150), np.float64)
    for j in range(150):
        gc[j:j + 11, j] = g

    fu = np.arange(1, 6, dtype=np.float64) / 5.0
    acol = np.zeros((160, 2, NP_, NU), np.float64)
    for pi in range(NP_):
        for a in range(12):
            for ui in range(NU):
                ang = -2.0 * math.pi * (ui + 1) * a / 12.0
                acol[4 * pi + a, 0, pi, ui] = math.cos(ang) * fu[ui]
                acol[4 * pi + a, 1, pi, ui] = math.sin(ang) * fu[ui]
    acol = acol.reshape(160, 380)

    bre = np.zeros((160, NP_, NU), np.float64)
    bim = np.zeros((160, NP_, NU), np.float64)
    for pj in range(NP_):
        for b in range(12):
            for vi in range(NU):
                ang = -2.0 * math.pi * (vi + 1) * b / 12.0
                bre[4 * pj + b, pj, vi] = math.cos(ang) * fu[vi]
                bim[4 * pj + b, pj, vi] = math.sin(ang) * fu[vi]
    bre = bre.reshape(160, 190)
    bim = bim.reshape(160, 190)

    return {
        "rhs1": np.concatenate([gc, acol], 1).astype(np.float32),
        "rhs2r": np.concatenate([bre, bim], 1).astype(np.float32),
        "rhs2i": np.concatenate([-bim, bre], 1).astype(np.float32),
        "fu190": np.tile(fu, NP_).astype(np.float32).reshape(190, 1),
        "fv": np.tile(fu, 3 * n).astype(np.float32).reshape(1, 15 * n),
    }


def build(n: int = 4, use_divide: bool = False, halves: int = 2):
    assert n % halves == 0
    nh = n // halves          # samples per half
    GH = 3 * nh               # chimg per half
    PW = GH * NFR             # packed width per half
    MW = nh * 450             # ssim map width per half
    G = 3 * n

    nc = bass.Bass("TRN2")

    x_in = nc.declare_dram_parameter("x_in", [n, H, H, C], F32, isOutput=False)
    x_out = nc.declare_dram_parameter("x_out", [n, H, H, C], F32, isOutput=False)
    mean = nc.declare_dram_parameter("mean", [n, 128], F32, isOutput=False)
    logvar = nc.declare_dram_parameter("logvar", [n, 128], F32, isOutput=False)
    rhs1_d = nc.declare_dram_parameter("rhs1", [160, 530], BF16, isOutput=False)
    rhs2r_d = nc.declare_dram_parameter("rhs2r", [160, 380], BF16, isOutput=False)
    rhs2i_d = nc.declare_dram_parameter("rhs2i", [160, 380], BF16, isOutput=False)
    fu190_d = nc.declare_dram_parameter("fu190", [190, 1], F32, isOutput=False)
    fv_d = nc.declare_dram_parameter("fv", [1, 15 * n], F32, isOutput=False)
    y_d = nc.declare_dram_parameter("y", [3, n], F32, isOutput=True)

    xs_d = {"in": x_in, "out": x_out}
    ctr = [0]

    with tile.TileContext(nc) as tc:
        with (
            tc.tile_pool(name="const", bufs=1) as cpool,
            tc.tile_pool(name="xt", bufs=2) as xpool,
            tc.tile_pool(name="prod", bufs=2) as ppool,
            tc.tile_pool(name="p1g", bufs=2, space="PSUM") as p1g,
            tc.tile_pool(name="p1s", bufs=2, space="PSUM") as p1s,
            tc.tile_pool(name="o1g", bufs=34) as o1g,
            tc.tile_pool(name="o1s", bufs=16) as o1s,
            tc.tile_pool(name="p2g", bufs=1, space="PSUM") as p2g,
            tc.tile_pool(name="maps", bufs=1) as mpool,
            tc.tile_pool(name="zps", bufs=2, space="PSUM") as zps,
            tc.tile_pool(name="zpack", bufs=1) as zpool,
            tc.tile_pool(name="esc", bufs=8) as esc,
            tc.tile_pool(name="fin", bufs=1) as fin,
            tc.tile_pool(name="finp", bufs=1, space="PSUM") as finp,
        ):
            # ---------------- constants ----------------
            rhs1b = {"a": cpool.tile([128, 530], BF16, tag="rhs1a", name="rhs1a"),
                     "c": cpool.tile([128, 530], BF16, tag="rhs1c", name="rhs1c")}
            nc.sync.dma_start(rhs1b["a"][:], rhs1_d[0:128, :])
            nc.sync.dma_start(rhs1b["c"][:], rhs1_d[32:160, :])
            r2r = {"a": cpool.tile([128, 380], BF16, tag="r2ra", name="r2ra"),
                   "c": cpool.tile([128, 380], BF16, tag="r2rc", name="r2rc")}
            r2i = {"a": cpool.tile([128, 380], BF16, tag="r2ia", name="r2ia"),
                   "c": cpool.tile([128, 380], BF16, tag="r2ic", name="r2ic")}
            nc.sync.dma_start(r2r["a"][:], rhs2r_d[0:128, :])
            nc.sync.dma_start(r2r["c"][:], rhs2r_d[32:160, :])
            nc.sync.dma_start(r2i["a"][:], rhs2i_d[0:128, :])
            nc.sync.dma_start(r2i["c"][:], rhs2i_d[32:160, :])
            fu_t = {0: cpool.tile([115, 1], F32, tag="fu0", name="fu0"),
                    1: cpool.tile([75, 1], F32, tag="fu1", name="fu1")}
            nc.sync.dma_start(fu_t[0][:], fu190_d[0:115, :])
            nc.sync.dma_start(fu_t[1][:], fu190_d[115:190, :])
            fv_t = cpool.tile([1, 15 * n], F32, tag="fv", name="fv")
            nc.sync.dma_start(fv_t[:], fv_d[:])
            ones = cpool.tile([128, 1], F32, tag="ones", name="ones")
            nc.gpsimd.memset(ones[:], 1.0)

            def evac(dst, src):
                ctr[0] += 1
                if ctr[0] % 2 == 0:
                    nc.scalar.copy(dst, src)
                else:
                    nc.vector.tensor_copy(dst, src)

            # ---------------- KLD ----------------
            mt = fin.tile([n, 128], F32, tag="mt", name="mt")
            lt = fin.tile([n, 128], F32, tag="lt", name="lt")
            nc.sync.dma_start(mt[:], mean[:])
            nc.sync.dma_start(lt[:], logvar[:])
            scr = fin.tile([n, 128], F32, tag="kscr", name="kscr")
            se = fin.tile([n, 1], F32, tag="se", name="se")
            sm = fin.tile([n, 1], F32, tag="sm", name="sm")
            sl = fin.tile([n, 1], F32, tag="sl", name="sl")
            nc.scalar.activation(scr[:], lt[:], ACTF.Exp, accum_out=se[:])
            nc.scalar.activation(scr[:], mt[:], ACTF.Square, accum_out=sm[:])
            nc.vector.tensor_reduce(sl[:], lt[:], AX.X, ALU.add)
            t1k = fin.tile([n, 1], F32, tag="t1k", name="t1k")
            nc.vector.tensor_tensor(t1k[:], sl[:], se[:], ALU.subtract)
            nc.vector.tensor_tensor(t1k[:], t1k[:], sm[:], ALU.subtract)
            kldn = fin.tile([n, 1], F32, tag="kldn", name="kldn")
            nc.vector.tensor_scalar(kldn[:], t1k[:], -0.5, -64.0, ALU.mult, ALU.add)

            # final psum accumulators packed in one bank:
            fa = finp.tile([1, 7 * G], F32, tag="fina", name="fina")
            argp = fa[:, 0:5 * G]
            ampp = fa[:, 5 * G:6 * G]
            ssimp = fa[:, 6 * G:7 * G]

            mapof = {"x": "mux", "y": "muy", "xx": "fxx", "yy": "fyy", "xy": "fxy"}
            Tof = {"x": "in", "y": "out"}
            MOFF = {"a": 0, "c": 32}

            for half in range(halves):
                s0 = half * nh
                zre, zim, rT, qT, mTl, thT = {}, {}, {}, {}, {}, {}
                for T in ("in", "out"):
                    for ci, (p0, p1) in enumerate(CH):
                        P = p1 - p0
                        zre[T, ci] = zpool.tile([P, PW], BF16, tag=f"zre{T}{ci}", name=f"zre{T}{ci}")
                        zim[T, ci] = zpool.tile([P, PW], BF16, tag=f"zim{T}{ci}", name=f"zim{T}{ci}")
                        rT[T, ci] = zpool.tile([P, PW], BF16, tag=f"r{T}{ci}", name=f"r{T}{ci}")
                        qT[T, ci] = zpool.tile([P, PW], BF16, tag=f"q{T}{ci}", name=f"q{T}{ci}")
                        mTl[T, ci] = zpool.tile([P, PW], BF16, tag=f"m{T}{ci}", name=f"m{T}{ci}")
                maps = {}
                for q in ("mux", "muy", "fxx", "fyy", "fxy"):
                    maps[q, 0] = mpool.tile([128, MW], BF16, tag=f"{q}0", name=f"{q}0")
                    maps[q, 1] = mpool.tile([22, MW], BF16, tag=f"{q}1", name=f"{q}1")

                # ============ heavy pipeline for this half ============
                for sl_ in range(nh):
                    s = s0 + sl_
                    xt = {}
                    for T in ("in", "out"):
                        for kb, h0 in (("a", 0), ("c", 32)):
                            t = xpool.tile([128, NPIX], BF16, tag=f"x{T}{kb}", name=f"x{T}{kb}")
                            nc.gpsimd.dma_start(
                                t[:], xs_d[T][s, h0:h0 + 128].rearrange("h w c -> h (w c)"))
                            xt[T, kb] = t

                    prods = {}
                    for pq, (ta, tb) in (("xx", ("in", "in")), ("yy", ("out", "out")),
                                         ("xy", ("in", "out"))):
                        for kb in ("a", "c"):
                            pt = ppool.tile([128, NPIX], BF16, tag=f"p{pq}{kb}", name=f"p{pq}{kb}")
                            nc.vector.tensor_tensor(pt[:], xt[ta, kb][:], xt[tb, kb][:],
                                                    ALU.mult)
                            prods[pq, kb] = pt

                    def lhs_tile(qn, kb, xt=xt, prods=prods):
                        if qn == "x":
                            return xt["in", kb]
                        if qn == "y":
                            return xt["out", kb]
                        return prods[qn, kb]

                    # ---- pass1 ----
                    o1g_t, o1s_t = {}, {}
                    def p1_one(qn, c, mb, o1g_t=o1g_t, o1s_t=o1s_t, lhs_tile=lhs_tile):
                        do_stft = qn in ("x", "y")
                        w0 = MOFF[mb]
                        pg = p1g.tile([128, J], F32, tag="pg", name="pg")
                        ps = p1s.tile([128, 380], F32, tag="ps", name="ps") if do_stft else None
                        for kb in ("a", "c"):
                            lhsT = lhs_tile(qn, kb).rearrange(
                                "p (w c) -> p w c", c=C)[:, w0:w0 + 128, c]
                            if kb == "a":
                                nc.tensor.matmul(pg[:, 0:JA], lhsT,
                                                 rhs1b["a"][:, 0:JA],
                                                 start=True, stop=True)
                            else:
                                nc.tensor.matmul(pg[:, JA:J], lhsT,
                                                 rhs1b["c"][:, JA:J],
                                                 start=True, stop=True)
                            if do_stft:
                                if kb == "a":
                                    nc.tensor.matmul(ps[:, 0:150], lhsT,
                                                     rhs1b["a"][:, 150:300],
                                                     start=True, stop=True)
                                    nc.tensor.matmul(ps[:, 190:340], lhsT,
                                                     rhs1b["a"][:, 340:490],
                                                     start=True, stop=True)
                                else:
                                    nc.tensor.matmul(ps[:, 150:190], lhsT,
                                                     rhs1b["c"][:, 300:340],
                                                     start=True, stop=True)
                                    nc.tensor.matmul(ps[:, 340:380], lhsT,
                                                     rhs1b["c"][:, 490:530],
                                                     start=True, stop=True)
                        og = o1g.tile([128, J], BF16, tag="og", name="og")
                        evac(og[:], pg[:])
                        o1g_t[qn, c, mb] = og
                        if do_stft:
                            os_ = o1s.tile([128, 380], BF16, tag="os", name="os")
                            evac(os_[:], ps[:])
                            o1s_t[Tof[qn], c, mb] = os_

                    for qn in ("x", "y", "xx", "yy", "xy"):
                        for c in range(C):
                            for mb in ("a", "c"):
                                p1_one(qn, c, mb)

                    # ---- pass2-gauss ----
                    def p2g_one(qn, ji, j0, j1, o1g_t=o1g_t, maps=maps, sl_=sl_):
                        po = p2g.tile([j1 - j0, 450], F32, tag="p2g", name=f"p2g{ji}")
                        for c in range(C):
                            for mb, jsl in (("a", (0, JA)), ("c", (JA, J))):
                                nc.tensor.matmul(
                                    po[:, c * J + jsl[0]:c * J + jsl[1]],
                                    o1g_t[qn, c, mb][:, j0:j1],
                                    rhs1b[mb][:, jsl[0]:jsl[1]],
                                    start=True, stop=True)
                        evac(maps[mapof[qn], ji][:, sl_ * 450:(sl_ + 1) * 450], po[:])

                    for qn in ("x", "y", "xx", "yy", "xy"):
                        for ji, (j0, j1) in enumerate(((0, 128), (128, J))):
                            p2g_one(qn, ji, j0, j1)

                    # ---- pass2-stft ----
                    def p2stft_one(T, c, ci, p0, p1, g, o1s_t=o1s_t, zre=zre, zim=zim):
                        P = p1 - p0
                        pzr = zps.tile([P, NFR], F32, tag="pz", name=f"pzr{ci}")
                        pzi = zps.tile([P, NFR], F32, tag="pz", name=f"pzi{ci}")
                        for beta, pz in (("r", pzr), ("i", pzi)):
                            bcol = 0 if beta == "r" else NFR
                            for mb in ("a", "c"):
                                lt_ = o1s_t[T, c, mb]
                                csl = (0, 150) if mb == "a" else (150, 190)
                                nc.tensor.matmul(
                                    pz[:, csl[0]:csl[1]],
                                    lt_[:, p0:p1],
                                    r2r[mb][:, bcol + csl[0]:bcol + csl[1]],
                                    start=True, stop=False)
                                nc.tensor.matmul(
                                    pz[:, csl[0]:csl[1]],
                                    lt_[:, NFR + p0:NFR + p1],
                                    r2i[mb][:, bcol + csl[0]:bcol + csl[1]],
                                    start=False, stop=True)
                        evac(zre[T, ci][:, g * NFR:(g + 1) * NFR], pzr[:])
                        evac(zim[T, ci][:, g * NFR:(g + 1) * NFR], pzi[:])

                    for T in ("in", "out"):
                        for c in range(C):
                            for ci, (p0, p1) in enumerate(CH):
                                p2stft_one(T, c, ci, p0, p1, sl_ * C + c)

                # ============ elementwise: phase A (sqrt set) ============
                for T in ("in", "out"):
                    for ci, (p0, p1) in enumerate(CH):
                        P = p1 - p0
                        zr, zi = zre[T, ci], zim[T, ci]
                        rr = esc.tile([P, PW], BF16, tag="e", name="rr")
                        ii = esc.tile([P, PW], BF16, tag="e", name="ii")
                        nc.scalar.activation(rr[:], zr[:], ACTF.Square)
                        nc.scalar.activation(ii[:], zi[:], ACTF.Square)
                        r2 = esc.tile([P, PW], BF16, tag="e", name="r2")
                        nc.vector.tensor_tensor(r2[:], rr[:], ii[:], ALU.add)
                        nc.scalar.activation(rT[T, ci][:], r2[:], ACTF.Sqrt)
                        rpx = esc.tile([P, PW], BF16, tag="e", name="rpx")
                        nc.vector.scalar_tensor_tensor(rpx[:], rT[T, ci][:], EPS,
                                                       zr[:], ALU.add, ALU.add)
                        pp = esc.tile([P, PW], BF16, tag="e", name="pp")
                        nc.scalar.activation(pp[:], rpx[:], ACTF.Square)
                        mx = esc.tile([P, PW], BF16, tag="e", name="mx")
                        nc.vector.scalar_tensor_tensor(mx[:], ii[:], 1e-30, pp[:],
                                                       ALU.max, ALU.max)
                        nc.vector.tensor_tensor(mTl[T, ci][:], ii[:], pp[:], ALU.is_gt)
                        prod = esc.tile([P, PW], BF16, tag="e", name="prod")
                        nc.vector.tensor_tensor(prod[:], zi[:], rpx[:], ALU.mult)
                        if use_divide:
                            nc.vector.tensor_tensor(qT[T, ci][:], prod[:], mx[:],
                                                    ALU.divide)
                        else:
                            inv = esc.tile([P, PW], F32, tag="ef", name="inv", bufs=2)
                            nc.vector.reciprocal(inv[:], mx[:])
                            nc.vector.tensor_tensor(qT[T, ci][:], prod[:], inv[:],
                                                    ALU.mult)

                # ============ phase B (trig set) + diffs + reduces ============
                for T in ("in", "out"):
                    for ci, (p0, p1) in enumerate(CH):
                        P = p1 - p0
                        u = esc.tile([P, PW], BF16, tag="e", name="u")
                        nc.scalar.activation(u[:], qT[T, ci][:], ACTF.Arctan)
                        yn = esc.tile([P, PW], BF16, tag="e", name="yn")
                        nc.vector.tensor_scalar(yn[:], zim[T, ci][:], 0.0, None,
                                                ALU.is_lt)
                        v2 = esc.tile([P, PW], BF16, tag="e", name="v2")
                        nc.vector.tensor_scalar(v2[:], yn[:], -2.0 * math.pi,
                                                math.pi, ALU.mult, ALU.add)
                        w1 = esc.tile([P, PW], BF16, tag="e", name="w1")
                        nc.vector.tensor_tensor(w1[:], mTl[T, ci][:], v2[:], ALU.mult)
                        t1 = esc.tile([P, PW], BF16, tag="e", name="t1")
                        nc.vector.tensor_tensor(t1[:], u[:], mTl[T, ci][:], ALU.mult)
                        nc.vector.scalar_tensor_tensor(t1[:], t1[:], -4.0, w1[:],
                                                       ALU.mult, ALU.add)
                        th = qT[T, ci]
                        nc.vector.scalar_tensor_tensor(th[:], u[:], 2.0, t1[:],
                                                       ALU.mult, ALU.add)
                        thT[T, ci] = th

                reds, redas = {}, {}
                for ci, (p0, p1) in enumerate(CH):
                    P = p1 - p0
                    d = esc.tile([P, PW], BF16, tag="e", name="d")
                    nc.vector.tensor_tensor(d[:], thT["out", ci][:], thT["in", ci][:],
                                            ALU.subtract)
                    red = esc.tile([P, 5 * GH], F32, tag="er", name="red")
                    nc.vector.tensor_reduce(
                        red[:], d.rearrange("p (g pj v) -> p g v pj", v=NU, pj=NP_),
                        AX.X, ALU.add, apply_absolute_value=True)
                    reds[ci] = red
                    da = esc.tile([P, PW], BF16, tag="e", name="da")
                    nc.vector.tensor_tensor(da[:], rT["out", ci][:], rT["in", ci][:],
                                            ALU.subtract)
                    reda = esc.tile([P, GH], F32, tag="er", name="reda")
                    nc.vector.tensor_reduce(
                        reda[:], da.rearrange("p (g f) -> p g f", f=NFR),
                        AX.X, ALU.add, apply_absolute_value=True)
                    redas[ci] = reda
                for ci, (p0, p1) in enumerate(CH):
                    nc.tensor.matmul(argp[:, half * 5 * GH:(half + 1) * 5 * GH],
                                     fu_t[ci][:], reds[ci][:],
                                     start=(ci == 0), stop=(ci == 1))
                for ci, (p0, p1) in enumerate(CH):
                    nc.tensor.matmul(ampp[:, half * GH:(half + 1) * GH],
                                     ones[0:p1 - p0, :], redas[ci][:],
                                     start=(ci == 0), stop=(ci == 1))

                # ============ ssim elementwise ============
                for ji, P in ((0, 128), (1, 22)):
                    mux, muy = maps["mux", ji], maps["muy", ji]
                    fxx, fyy, fxy = maps["fxx", ji], maps["fyy", ji], maps["fxy", ji]
                    mxy = esc.tile([P, MW], BF16, tag="e", name="smxy")
                    nc.vector.tensor_tensor(mxy[:], mux[:], muy[:], ALU.mult)
                    mx2 = esc.tile([P, MW], BF16, tag="e", name="smx2")
                    nc.scalar.activation(mx2[:], mux[:], ACTF.Square)
                    my2 = esc.tile([P, MW], BF16, tag="e", name="smy2")
                    nc.scalar.activation(my2[:], muy[:], ACTF.Square)
                    s12 = esc.tile([P, MW], BF16, tag="e", name="ss12")
                    nc.vector.tensor_tensor(s12[:], mx2[:], my2[:], ALU.add)
                    vxy = esc.tile([P, MW], BF16, tag="e", name="svxy")
                    nc.vector.tensor_tensor(vxy[:], fxx[:], fyy[:], ALU.add)
                    nc.vector.tensor_tensor(vxy[:], vxy[:], s12[:], ALU.subtract)
                    cov = esc.tile([P, MW], BF16, tag="e", name="scov")
                    nc.vector.tensor_tensor(cov[:], fxy[:], mxy[:], ALU.subtract)
                    n1 = esc.tile([P, MW], BF16, tag="e", name="sn1")
                    nc.vector.tensor_scalar(n1[:], mxy[:], 2.0, C1, ALU.mult, ALU.add)
                    n2 = esc.tile([P, MW], BF16, tag="e", name="sn2")
                    nc.vector.tensor_scalar(n2[:], cov[:], 2.0, C2, ALU.mult, ALU.add)
                    d1 = esc.tile([P, MW], BF16, tag="e", name="sd1")
                    nc.vector.tensor_scalar(d1[:], s12[:], C1, None, ALU.add)
                    d2 = esc.tile([P, MW], BF16, tag="e", name="sd2")
                    nc.vector.tensor_scalar(d2[:], vxy[:], C2, None, ALU.add)
                    nn = esc.tile([P, MW], BF16, tag="e", name="snn")
                    nc.vector.tensor_tensor(nn[:], n1[:], n2[:], ALU.mult)
                    dd = esc.tile([P, MW], F32, tag="ef", name="sdd", bufs=2)
                    nc.vector.tensor_tensor(dd[:], d1[:], d2[:], ALU.mult)
                    idd = esc.tile([P, MW], F32, tag="ef", name="sidd", bufs=2)
                    nc.vector.reciprocal(idd[:], dd[:])
                    val = esc.tile([P, MW], BF16, tag="e", name="sval")
                    nc.vector.tensor_tensor(val[:], nn[:], idd[:], ALU.mult)
                    sred = esc.tile([P, GH], F32, tag="er", name="sred")
                    nc.vector.tensor_reduce(
                        sred[:], val.rearrange("p (sc j2) -> p sc j2", j2=J),
                        AX.X, ALU.add)
                    nc.tensor.matmul(ssimp[:, half * GH:(half + 1) * GH],
                                     ones[0:P, :], sred[:],
                                     start=(ji == 0), stop=(ji == 1))

            # ---------------- final assembly ----------------
            argv = fin.tile([1, 5 * G], F32, tag="argv", name="argv")
            nc.vector.tensor_tensor(argv[:], argp, fv_t[:], ALU.mult)
            arg12 = fin.tile([1, G], F32, tag="arg12", name="arg12")
            nc.vector.tensor_reduce(
                arg12[:], argv.rearrange("p (g v) -> p g v", v=NU), AX.X, ALU.add)
            st12 = fin.tile([1, G], F32, tag="st12", name="st12")
            nc.vector.tensor_tensor(st12[:], arg12[:], ampp, ALU.add)
            stn = fin.tile([1, n], F32, tag="stn", name="stn")
            nc.vector.tensor_reduce(
                stn[:], st12.rearrange("p (s c) -> p s c", c=C), AX.X, ALU.add)
            ssn = fin.tile([1, n], F32, tag="ssn", name="ssn")
            nc.vector.tensor_reduce(
                ssn[:], ssimp.rearrange("p (s c) -> p s c", c=C), AX.X, ALU.add)
            kldT = fin.tile([1, n], F32, tag="kldT", name="kldT")
            nc.sync.dma_start(kldT[:], kldn[:])
            nc.sync.dma_start(y_d[0:1, :], kldT[:])
            nc.sync.dma_start(y_d[1:2, :], ssn[:])
            nc.sync.dma_start(y_d[2:3, :], stn[:])

    return nc


# ======================================================================
# Walrus single-sync-wait workarounds (see tile_patch rationale above)
# ======================================================================


import bass_rust
import concourse.mybir as mybir
from concourse import tile as _tile_mod
from concourse.tile import TileContext

_UNASSIGNED = mybir.EngineType.Unassigned


def _patched_drain_and_barrier(self, tick_clock, wait_clock):
    nc = self.nc
    drain_inst = nc.sync.drain()
    wait_clock.add_sem_waits(
        drain_inst.ins, _tile_mod.ScopedClock({None: tick_clock.global_clock})
    )
    si = drain_inst.ins.sync_info
    if si is not None and si.on_wait and len(si.on_wait) > 1:
        waits = list(si.on_wait)
        drain_inst.ins.sync_info = bass_rust.SyncInfo(
            on_wait=[waits[0]], on_update=list(si.on_update or [])
        )
        for w in waits[1:]:
            d2 = nc.sync.drain()
            d2.ins.sync_info = bass_rust.SyncInfo(on_wait=[w], on_update=[])

    nc.all_engine_barrier()
    assert self.sems is not None
    popped = nc._tile_sem_poison_stack.pop()
    assert popped is self._sem_poison
    nc.clear_and_free_semaphores(list(self.sems.allocated().values()))
    nc.all_engine_barrier()


_orig_commit = TileContext._commit_instruction


def _patched_commit(self, inst, lazy_reg_writes: bool = True):
    si = inst.sync_info
    if (
        si is not None
        and si.on_wait
        and len(si.on_wait) > 1
        and inst.engine != _UNASSIGNED
    ):
        waits = list(si.on_wait)
        inst.sync_info = bass_rust.SyncInfo(
            on_wait=[waits[-1]], on_update=list(si.on_update or [])
        )
        for w in waits[:-1]:
            nop = mybir.InstNoOp(
                name=self.nc.get_next_instruction_name(), ins=[], outs=[]
            )
            nop.engine = inst.engine
            nop.sync_info = bass_rust.SyncInfo(on_wait=[w], on_update=[])
            self._add_instruction(nop)
    return _orig_commit(self, inst, lazy_reg_writes)


TileContext._drain_and_barrier = _patched_drain_and_barrier
TileContext._commit_instruction = _patched_commit


# ======================================================================
# Host-side entry point: full inputs in, full output out (8-core SPMD).
# ======================================================================

import ml_dtypes
from concourse.bass_utils import run_bass_kernel_spmd

N_CORES = 8
_cache = {}


def _get_nc(nper):
    if nper not in _cache:
        _cache[nper] = build(nper)
    return _cache[nper]


def run_spmd(mean, logvar, x_in, x_out, **spmd_kwargs):
    B = x_in.shape[0]
    nper = B // N_CORES
    nc = _get_nc(nper)
    consts = make_consts(nper)
    bf = ml_dtypes.bfloat16
    cfeed = {
        "rhs1": consts["rhs1"].astype(bf),
        "rhs2r": consts["rhs2r"].astype(bf),
        "rhs2i": consts["rhs2i"].astype(bf),
        "fu190": consts["fu190"],
        "fv": consts["fv"],
    }
    in_maps = []
    for i in range(N_CORES):
        sl = slice(i * nper, (i + 1) * nper)
        m = {"x_in": np.ascontiguousarray(x_in[sl]),
             "x_out": np.ascontiguousarray(x_out[sl]),
             "mean": np.ascontiguousarray(mean[sl]),
             "logvar": np.ascontiguousarray(logvar[sl])}
        m.update(cfeed)
        in_maps.append(m)
    return run_bass_kernel_spmd(nc, in_maps, list(range(N_CORES)), **spmd_kwargs)


def kernel(mean, logvar, x_in, x_out):
    res = run_spmd(np.asarray(mean, np.float32), np.asarray(logvar, np.float32),
                   np.asarray(x_in, np.float32), np.asarray(x_out, np.float32))
    per_sample = []
    for r in res.results:
        y = np.asarray(r["y"], np.float32)   # [3, nper]
        per_sample.append(y[0] + y[1] / 67500.0 + 1e-4 * y[2])
    return np.float32(np.mean(np.concatenate(per_sample)))

